# revision 84
# baseline (speedup 1.0000x reference)
"""Fused pre-LN multi-head attention block for Trainium2, 8-core data parallel.

Problem (per batch element, B=8, one batch element per core):
  x: [2048, 128] f32
  pre-LN -> QKV -> heads via a RAW reshape (B,S,D)->(B,H,S',dh): head h is a
  256-token slab; its 2048 "subtokens" are (token, feature-group) pairs:
  subtoken 8*t+g of head h = features [16g:16g+16) of token 256h+t.
  energy = Qv @ Kv^T per head; softmax; /sqrt(128) AFTER softmax; O = A @ Vv;
  raw reshape back; out = O @ Wo^T + x.

v4 design (pipeline-overlap driven; sim 290us -> 190us vs the v2 baseline):
  - Subtokens processed in block-major kappa order (kappa = 256*g + t), legal
    because softmax+AV are k-permutation invariant and the q-permutation is
    undone at output assembly.
  - Q/K stacks via natural-layout Q^T/K^T half-tensors: matmuls produce
    QT_half [4 g-blocks x (16 data + 16 zero) rows, tokens] (full-width
    PSUM->SBUF copies, bf16), then the (g: partition -> free) band shuffle
    runs as 32-aligned [32, 256] SBUF->SBUF copies on the otherwise-idle
    Pool engine (head 0's copies on Act/DVE for early start). This removes
    the 32-partition-wide stack copies (~30us of Act/DVE) of the v2 design;
    the zero pad rows ride along in the 32-row copies for free.
  - Energy: bf16 matmuls 512 wide (1 cycle/row), lhsT = KST 32-row
    zero-padded blocks via tile_position row banding.
  - exp is the wall (8*2048^2 = 33.5M elements/core on two engines): split
    between Act (true Exp) and DVE (Schraudolph exp: bf16 bits =
    int16(x*128/ln2 + (127-c)*128), ~3% rel err, irrelevant after softmax
    normalization) in 1024-col chunks with a 3-deep PSUM ping-pong (6 banks)
    so both engines run concurrently and the PE never waits for a free bank.
    A greedy balancer with tuned per-instruction fixed costs (Act ~365ns:
    222-cycle SBUF access + seq dispatch with exec-queue depth 0; DVE
    ~200ns with its depth-8 exec queue) assigns every flexible op; the
    equilibrium runs DVE ~88% busy and Act ~81%.
  - AV flipped: A (SBUF, bf16) is the *stationary* operand, the 17-wide
    [V|1] block is moving (17 rows/matmul, output in natural q orientation).
  - Ones column rides the AV matmul for deferred softmax normalization;
    1/sqrt(128) is folded into Wo host-side. Finalize normalizes both
    par-halves with a single reciprocal + tensor_tensor per head.
  - LayerNorm stats via bn_stats/bn_aggr (DVE), xn application on the idle
    Pool engine; x DMA'd in 4 per-LN-group tiles issued ahead of the weight
    DMAs so the first energy matmul lands ~6.5us in. Second-half LN, Q^T/K^T,
    X=1 shuffles and V projections drip into the first main-loop units.
  - AV lags exp by 3 units and is emitted before each unit's exp chunks, so
    the drain after the last exp is one AV burst + one finalize + DMA out.
"""

import sys

if "/opt/trn_rl_repo" not in sys.path:
    sys.path.insert(0, "/opt/trn_rl_repo")

from contextlib import ExitStack

import numpy as np

import concourse.bacc as bacc
import concourse.mybir as mybir
from concourse import masks
from concourse.tile import TileContext
from concourse.bass_utils import run_bass_kernel_spmd

S = 2048
D = 128
H = 8
DH = 16
NT = S // 128           # 16 s-tiles / k-tiles per batch element
VBLK = 17 * H           # 136 cols per s-tile block in v_sb
NQC = 4                 # 512-wide q chunks per head
NCH = 8                 # 1024-col exp chunks per unit (2 k-tiles each)
EPS = 1e-5
F32 = mybir.dt.float32
F32R = mybir.dt.float32r
BF16 = mybir.dt.bfloat16
I16 = mybir.dt.int16
AX = mybir.AluOpType
AF = mybir.ActivationFunctionType

LN2 = float(np.log(2.0))
SCH_A = 128.0 / LN2
SCH_B = (127.0 - 0.0579) * 128.0


class Balancer:
    """Greedy Act/DVE assignment for flexible ops (exp chunks, psum copies)."""

    ACT_NS = 0.8333
    DVE_NS = 1.0417

    def __init__(self):
        self.t = {"act": 0.0, "dve": 0.0}

    def charge(self, eng, cols, fixed=150.0):
        rate = self.ACT_NS if eng == "act" else self.DVE_NS
        self.t[eng] += cols * rate + fixed

    def pick(self, cols, dve_cols=None):
        fin_act = self.t["act"] + cols * self.ACT_NS + 365.0
        fin_dve = (
            self.t["dve"]
            + (cols if dve_cols is None else dve_cols) * self.DVE_NS
            + 200.0
        )
        if fin_act <= fin_dve:
            self.t["act"] = fin_act
            return "act"
        self.t["dve"] = fin_dve
        return "dve"


def build_mha_kernel(tc, outs, ins):
    nc = tc.nc
    x_d = ins["x"]          # [2048, 128] natural
    wv2_d = ins["wv2"]      # [128, 256] = [(Wv*ln_w).T | dup]
    wq2_d = ins["wq2"]      # [128, 256] 2 halves x 4 zero-padded g-blocks
    wk2_d = ins["wk2"]      # [128, 256] same for Wk
    wo_d = ins["wo"]        # [128, 128] = Wo.T / sqrt(D)
    out_d = outs["out_t"]   # [128, 2048] out^T (residual added host-side)

    bal = Balancer()

    ctx = ExitStack()
    const = ctx.enter_context(tc.tile_pool(name="const", bufs=1))
    iop = ctx.enter_context(tc.tile_pool(name="io", bufs=1))
    workp = ctx.enter_context(tc.tile_pool(name="work", bufs=2))
    psum = ctx.enter_context(tc.tile_pool(name="psum", bufs=1, space="PSUM"))

    def flex_copy(out_ap, in_ap, cols, dve_cols=None):
        # dve_cols: effective DVE cost override (bf16 SBUF->SBUF copies run
        # in the 4x_2p DVE mode, so their DVE cost is cols/4)
        if bal.pick(cols, dve_cols) == "act":
            nc.scalar.activation(out_ap, in_ap, AF.Copy)
        else:
            nc.vector.tensor_copy(out_ap, in_ap)

    def flex_exp(out_ap, in_ap, cols, force=None):
        eng = force if force is not None else bal.pick(cols)
        if force is not None:
            bal.charge(force, cols)
        if eng == "act":
            nc.scalar.activation(out_ap, in_ap, AF.Exp)
        else:
            nc.vector.tensor_scalar(
                out=out_ap.bitcast(I16), in0=in_ap,
                scalar1=SCH_A, scalar2=SCH_B, op0=AX.mult, op1=AX.add,
            )

    # ---- big persistent SBUF tensors ---------------------------------------
    # x natural [p, (t d)], split into 4 tiles so LayerNorm group q depends
    # only on its own DMA chunk; x DMAs are issued FIRST so the weight DMAs
    # don't serialize ahead of them on the HWDGE queue.
    x_sb4 = [
        iop.tile([128, 512], F32, tag=f"x{q}", name=f"x{q}") for q in range(4)
    ]
    for q in range(4):
        nc.sync.dma_start(
            out=x_sb4[q][:].rearrange("p (t d) -> p t d", d=D),
            in_=x_d.rearrange("(t p) d -> p t d", p=128)[:, 4 * q:4 * q + 4, :],
        )

    def x_tile(st):
        return x_sb4[st // 4][:, (st % 4) * D:(st % 4 + 1) * D]

    # ---- constants / weights (DMA'd after x; first needed ~7us in) --------
    ident = const.tile([128, 128], F32, tag="ident")
    masks.make_identity(nc, ident[:])
    ident_r = const.tile([128, 128], F32R, tag="identr")
    nc.vector.tensor_copy(ident_r[:], ident[:])
    wq2_sb = const.tile([128, 256], F32R, tag="wq2")
    nc.sync.dma_start(out=wq2_sb[:], in_=wq2_d[:])
    wk2_sb = const.tile([128, 256], F32R, tag="wk2")
    nc.sync.dma_start(out=wk2_sb[:], in_=wk2_d[:])
    wv2_sb = const.tile([128, 256], F32R, tag="wv2")
    nc.sync.dma_start(out=wv2_sb[:], in_=wv2_d[:])
    wo_sb = const.tile([128, 128], F32R, tag="wo")
    nc.sync.dma_start(out=wo_sb[:], in_=wo_d[:])
    wob_sb = const.tile([128, 128], BF16, tag="wob")
    nc.gpsimd.tensor_copy(wob_sb[:], wo_sb[:])
    xnt_sb = iop.tile([128, S], F32R, tag="xnt")      # xn^T   [d, s]
    v_sb = iop.tile([128, NT * VBLK], BF16, tag="v")  # [V|1] blocks per s-tile
    qst = [iop.tile([128, S], BF16, tag=f"qst{i}", name=f"qst{i}") for i in range(2)]
    kst = [iop.tile([128, S], BF16, tag=f"kst{i}", name=f"kst{i}") for i in range(2)]
    # Q^T/K^T halves: [4 g-blocks x (16 ch + 16 zero) rows, 2048 tokens]
    qth = [iop.tile([128, S], BF16, tag=f"qth{i}", name=f"qth{i}") for i in range(2)]
    kth = [iop.tile([128, S], BF16, tag=f"kth{i}", name=f"kth{i}") for i in range(2)]
    otT_sb = iop.tile([128, S], BF16, tag="otT")      # O^T (normalized)
    outt_sb = iop.tile([128, S], F32, tag="outt")     # final out^T

    ones_sb = const.tile([128, 8], F32, tag="ones")
    nc.gpsimd.memset(ones_sb[:], 1.0)

    def emit_ones():
        # v_sb ones columns; deferred so the Pool stream reaches the first
        # xn applications before this (only needed by the first AV, iter 2)
        for st in range(NT):
            nc.gpsimd.tensor_copy(
                v_sb[:, st * VBLK:(st + 1) * VBLK]
                .rearrange("p (g c) -> p g c", c=17)[:, :, 16:17],
                ones_sb[:].rearrange("p (g c) -> p g c", c=1),
            )

    # ---- LayerNorm (stats on DVE via bn_stats, apply on Pool) + xn^T -------
    stats = const.tile([128, 64], F32, tag="stats")
    mv = stats[:, 0:32].rearrange("p (t c) -> p t c", c=2)   # (mean, var)
    rstd = stats[:, 32:48]
    mrs = stats[:, 48:64]

    def emit_ln_stats(g4):
        s4 = slice(4 * g4, 4 * g4 + 4)
        bn6 = workp.tile([128, 24], F32, tag="bn6", bufs=2)
        for j in range(4):
            st = 4 * g4 + j
            nc.vector.bn_stats(
                out=bn6[:, 6 * j:6 * (j + 1)],
                in_=x_tile(st),
            )
            bal.charge("dve", 128)
            nc.vector.bn_aggr(out=mv[:, st, :], in_=bn6[:, 6 * j:6 * (j + 1)])
            bal.charge("dve", 6)
        # rstd = sqrt(1/(var+eps)); mrs = mean*rstd
        var_ap = mv[:, s4, 1:2]
        nc.vector.tensor_scalar_add(rstd[:, s4], var_ap.opt(), EPS)
        nc.vector.reciprocal(rstd[:, s4], rstd[:, s4])
        bal.charge("dve", 8)
        nc.scalar.activation(rstd[:, s4], rstd[:, s4], AF.Sqrt)
        bal.charge("act", 4)
        nc.gpsimd.tensor_mul(mrs[:, s4], mv[:, s4, 0:1].opt(), rstd[:, s4])

    def emit_ln_apply(g4):
        pt = psum.tile([128, 512], F32R, tag="e", bufs=3)
        for j in range(4):
            st = 4 * g4 + j
            xn = workp.tile([128, 128], F32R, tag="xn", bufs=8)
            nc.gpsimd.tensor_scalar(
                out=xn[:], in0=x_tile(st),
                scalar1=rstd[:, st:st + 1], scalar2=mrs[:, st:st + 1],
                op0=AX.mult, op1=AX.subtract,
            )
            nc.tensor.transpose(pt[:, j * 128:(j + 1) * 128], xn[:], ident_r[:])
        flex_copy(xnt_sb[:, g4 * 512:(g4 + 1) * 512], pt[:], 512)

    def emit_ln(g4):
        emit_ln_stats(g4)
        emit_ln_apply(g4)

    # ---- Q^T/K^T half-tensor production ------------------------------------
    # One matmul per (side, half, 512-token chunk): out rows 32a+c =
    # channel 16(4*half+a)+c for c<16, zero for c>=16.
    def emit_qt(side, half, tp):
        # tp: token pair index (0: tokens 0..1023, 1: 1024..2047)
        w_sb, dst = (wq2_sb, qth) if side == "q" else (wk2_sb, kth)
        qt = psum.tile([128, 1024], F32, tag="e", bufs=3, name="qt")
        for j in range(2):
            tok0 = tp * 1024 + j * 512
            nc.tensor.matmul(
                qt[:, j * 512:(j + 1) * 512],
                lhsT=w_sb[:, half * 128:(half + 1) * 128],
                rhs=xnt_sb[:, tok0:tok0 + 512],
                start=True, stop=True,
            )
        flex_copy(dst[half][:, tp * 1024:(tp + 1) * 1024], qt[:], 1024)

    # band shuffle: (qst|kst)[X][32m:32m+32, 256g:+256] =
    #               (qth|kth)[g//4][32(g%4):+32, 256(4X+m):+256]
    # (zero pad rows ride along). Pool for all but head 0 (early start).
    def emit_shuffle(X, m, side, g, on_pool):
        src = (qth if side == "q" else kth)[g // 4][
            32 * (g % 4):32 * (g % 4) + 32,
            256 * (4 * X + m):256 * (4 * X + m) + 256,
        ]
        dst = (qst if side == "q" else kst)[X][
            32 * m:32 * m + 32, 256 * g:256 * (g + 1)
        ]
        if on_pool:
            nc.gpsimd.tensor_copy(dst, src)
        else:
            flex_copy(dst, src, 256, dve_cols=64)

    def emit_v(sp):
        # two s-tiles' V projections in one PSUM tile, one strided copy
        vp = psum.tile([128, 512], F32, tag="e", bufs=3, name="vp")
        for j in range(2):
            st = 2 * sp + j
            nc.tensor.matmul(
                vp[:, j * 256:(j + 1) * 256],
                lhsT=xnt_sb[:, st * 128:(st + 1) * 128],
                rhs=wv2_sb[:],
                start=True, stop=True,
            )
        flex_copy(
            v_sb[:, 2 * sp * VBLK:(2 * sp + 2) * VBLK]
            .rearrange("p (s g c) -> p s g c", s=2, c=17)[:, :, :, 0:16],
            vp[:].rearrange("p (s d g c) -> p s d g c", s=2, d=2, c=16)
            [:, :, 0, :, :],
            256,
        )

    # ---- preamble orchestration --------------------------------------------
    # Upfront: only what unit 0 (head 0) needs — LN of tokens 0..1023,
    # Q^T/K^T for those tokens, head 0's shuffle (flex engines), V0, and the
    # X0 m1-3 Pool shuffles (Pool doesn't block the exp engines). LN of the
    # second half, Q^T/K^T-B, X1 Pool shuffles, and V1..7 drip into the
    # first main-loop units (deadline: head m of X at unit 4*(4X+m)).
    emit_ln(0)
    emit_ln(1)
    emit_ones()
    for half in range(2):
        emit_qt("k", half, 0)
    for half in range(2):
        emit_qt("q", half, 0)
    for g in range(H):
        emit_shuffle(0, 0, "k", g, on_pool=False)
    for g in range(H):
        emit_shuffle(0, 0, "q", g, on_pool=False)
    emit_v(0)
    for m in range(1, 4):
        for g in range(H):
            emit_shuffle(0, m, "k", g, on_pool=True)
        for g in range(H):
            emit_shuffle(0, m, "q", g, on_pool=True)

    def emit_qtb_shuffles():
        for m in range(4):
            for g in range(H):
                emit_shuffle(1, m, "k", g, on_pool=True)
            for g in range(H):
                emit_shuffle(1, m, "q", g, on_pool=True)

    late_work = (
        [("ln", 2), ("ln", 3),
         ("qt", "k", 0, 1), ("qt", "k", 1, 1),
         ("qt", "q", 0, 1), ("qt", "q", 1, 1),
         ("shufB",), ("v", 1)]
        + [("v", sp) for sp in range(2, 8)]
    )

    def emit_late(w):
        if w[0] == "ln":
            emit_ln(w[1])
        elif w[0] == "qt":
            emit_qt(w[1], w[2], w[3])
        elif w[0] == "shufB":
            emit_qtb_shuffles()
        else:
            emit_v(w[1])

    # ---- attention main loop (software-pipelined: AV lags E/exp by 3) ------
    av_tile = {}
    pend = []

    def emit_e_exp(h, qc, last=False):
        X, m = h // 4, h % 4
        a_sb = workp.tile([128, NT * 512], BF16, tag="a", bufs=8)
        for c in range(NCH):
            et = psum.tile([128, 1024], F32, tag="e", bufs=3)
            for j in range(2):
                kt = 2 * c + j
                nc.tensor.matmul(
                    et[:, j * 512:(j + 1) * 512],
                    lhsT=kst[X][32 * m:32 * m + 32, kt * 128:(kt + 1) * 128],
                    rhs=qst[X][32 * m:32 * m + 32,
                               qc * 512:(qc + 1) * 512],
                    start=True, stop=True,
                    tile_position=(32 * m, 0),
                )
            # on the very last unit, pin the final two chunks to opposite
            # engines so the drain (AV -> finalize -> DMA) starts earliest
            force = None
            if last and c >= NCH - 2:
                force = "dve" if c == NCH - 2 else "act"
            flex_exp(a_sb[:, c * 1024:(c + 1) * 1024], et[:], 1024, force)
        return a_sb

    def emit_av(h, qc, a_sb):
        if qc == 0:
            av_tile[h] = psum.tile([128, 512], F32, tag="av", bufs=2, name="avbank")
        bank = av_tile[h]
        for qb in range(4):
            slot = 4 * qc + qb
            for kt in range(NT):
                st = 2 * h + (kt % 2)
                gk = kt // 2
                nc.tensor.matmul(
                    bank[:, slot * 17:slot * 17 + 17],
                    lhsT=a_sb[:, kt * 512 + qb * 128:kt * 512 + (qb + 1) * 128],
                    rhs=v_sb[:, st * VBLK + 17 * gk:st * VBLK + 17 * gk + 17],
                    start=(kt == 0), stop=(kt == NT - 1),
                )

    def emit_finalize(h):
        # split=True (last head): two independent per-par chains so the
        # copy -> Wo -> copy -> DMA latency pipelines at the kernel tail.
        split = False
        bank = av_tile.pop(h)
        # one reciprocal + one normalize covering BOTH par-halves; the output
        # AP de-interleaves slots so each par's 128 cols land contiguous
        slots = bank[:, 0:272].rearrange("p (q par c) -> p q par c", par=2, c=17)
        rec = workp.tile([128, 16], F32, tag="rec", bufs=2)
        nc.vector.reciprocal(rec[:], slots[:, :, :, 16:17].opt())
        bal.charge("dve", 16)
        o_nat = workp.tile([128, 256], F32R, tag="on", bufs=2)
        nc.vector.tensor_tensor(
            out=o_nat[:].rearrange("p (par q c) -> p q par c", par=2, c=DH),
            in0=slots[:, :, :, 0:16],
            in1=rec[:].rearrange("p (q par) -> p q par", par=2)
            .broadcast_to((128, 8, 2, DH)),
            op=AX.mult,
        )
        bal.charge("dve", 256)
        onats = [o_nat[:, 0:128], o_nat[:, 128:256]]
        fin = psum.tile([128, 512], F32R, tag="av", bufs=2, name="fin")

        def tail(lo, hi):
            cols = hi - lo
            o0 = 256 * h + lo
            flex_copy(otT_sb[:, o0:o0 + cols], fin[:, lo:lo + cols], cols)
            nc.tensor.matmul(
                fin[:, 256 + lo:256 + hi].bitcast(F32),
                lhsT=wob_sb[:],
                rhs=otT_sb[:, o0:o0 + cols],
                start=True, stop=True,
            )
            flex_copy(outt_sb[:, o0:o0 + cols],
                      fin[:, 256 + lo:256 + hi].bitcast(F32), cols)
            nc.sync.dma_start(
                out=out_d[:, o0:o0 + cols],
                in_=outt_sb[:, o0:o0 + cols],
            )

        if split:
            for par in range(2):
                nc.tensor.transpose(
                    fin[:, par * 128:(par + 1) * 128], onats[par], ident_r[:]
                )
                tail(par * 128, (par + 1) * 128)
        else:
            for par in range(2):
                nc.tensor.transpose(
                    fin[:, par * 128:(par + 1) * 128], onats[par], ident_r[:]
                )
            tail(0, 256)

    for u in range(H * NQC):
        h, qc = divmod(u, NQC)
        # AV lags exp by 2; emitted BEFORE the unit's exp so finalize's
        # DVE work queues ahead of the next exp chunks (frees the av bank
        # sooner for the next head's accumulator).
        if len(pend) >= 4:
            u2 = pend.pop(0)
            emit_av(*u2)
            if u2[1] == NQC - 1:
                emit_finalize(u2[0])
        a_sb = emit_e_exp(h, qc, last=(u == H * NQC - 1))
        # drip-feed two late-preamble pieces per early unit
        for _ in range(2):
            if late_work:
                emit_late(late_work.pop(0))
        pend.append((h, qc, a_sb))
    while pend:
        u2 = pend.pop(0)
        emit_av(*u2)
        if u2[1] == NQC - 1:
            emit_finalize(u2[0])

    ctx.close()


def host_prep(x, ln_w, ln_b, Wq, bq, Wk, bk, Wv, bv, Wo, bo):
    """Fold LN affine into weights; build combined zero-padded Q/K g-blocks."""
    for name, b in (("ln_b", ln_b), ("bq", bq), ("bk", bk), ("bv", bv), ("bo", bo)):
        assert np.abs(np.asarray(b)).max() == 0.0, f"{name} must be zero"
    ln_w = np.asarray(ln_w, np.float32)
    Wq_e = np.asarray(Wq, np.float32) * ln_w[None, :]
    Wk_e = np.asarray(Wk, np.float32) * ln_w[None, :]
    Wv_e = np.asarray(Wv, np.float32) * ln_w[None, :]

    # [128, 2*128]: half h, block a = [W.T cols 16(4h+a):+16 | 16 zeros]
    def halves(W):
        out = np.zeros((D, 256), np.float32)
        WT = W.T
        for h in range(2):
            for a in range(4):
                g = 4 * h + a
                out[:, h * 128 + a * 32:h * 128 + a * 32 + 16] = WT[
                    :, 16 * g:16 * (g + 1)
                ]
        return out

    wq2 = halves(Wq_e)
    wk2 = halves(Wk_e)
    wv2 = np.concatenate([Wv_e.T, Wv_e.T], axis=1).astype(np.float32)
    wv2 = np.ascontiguousarray(wv2)
    wo = np.ascontiguousarray(np.asarray(Wo, np.float32).T / np.sqrt(np.float32(D)))
    return wv2, wq2, wk2, wo


_CACHED = {}


def _build_nc():
    nc = bacc.Bacc("TRN2", target_bir_lowering=False, debug=False, num_devices=8)
    x_in = nc.dram_tensor("x", [S, D], F32, kind="ExternalInput").ap()
    wv2_in = nc.dram_tensor("wv2", [D, 256], F32R, kind="ExternalInput").ap()
    wq2_in = nc.dram_tensor("wq2", [D, 256], F32R, kind="ExternalInput").ap()
    wk2_in = nc.dram_tensor("wk2", [D, 256], F32R, kind="ExternalInput").ap()
    wo_in = nc.dram_tensor("wo", [D, D], F32R, kind="ExternalInput").ap()
    out_t = nc.dram_tensor("out_t", [D, S], F32, kind="ExternalOutput").ap()
    with TileContext(nc) as tc:
        build_mha_kernel(
            tc,
            {"out_t": out_t},
            {"x": x_in, "wv2": wv2_in, "wq2": wq2_in, "wk2": wk2_in,
             "wo": wo_in},
        )
    nc.compile()
    return nc


def kernel(x, ln_w, ln_b, Wq, bq, Wk, bk, Wv, bv, Wo, bo):
    x = np.asarray(x, np.float32)
    B = x.shape[0]
    assert x.shape == (B, S, D) and B == 8
    wv2, wq2, wk2, wo = host_prep(x, ln_w, ln_b, Wq, bq, Wk, bk, Wv, bv, Wo, bo)

    if "nc" not in _CACHED:
        _CACHED["nc"] = _build_nc()
    nc = _CACHED["nc"]

    in_maps = [
        {
            "x": np.ascontiguousarray(x[c]),
            "wv2": wv2,
            "wq2": wq2,
            "wk2": wk2,
            "wo": wo,
        }
        for c in range(B)
    ]
    res = run_bass_kernel_spmd(nc, in_maps, core_ids=list(range(B)))
    out = np.stack([res.results[c]["out_t"].T for c in range(B)]) + x
    return out.astype(np.float32)


if __name__ == "__main__":
    rng = np.random.default_rng(0)
    x = rng.standard_normal((8, S, D), dtype=np.float32)
    ln_w = np.ones(D, np.float32)
    z = np.zeros(D, np.float32)
    s = 1.0 / np.sqrt(D)
    Wq = rng.standard_normal((D, D), dtype=np.float32) * s
    Wk = rng.standard_normal((D, D), dtype=np.float32) * s
    Wv = rng.standard_normal((D, D), dtype=np.float32) * s
    Wo = rng.standard_normal((D, D), dtype=np.float32) * s
    out = kernel(x, ln_w, z, Wq, z, Wk, z, Wv, z, Wo, z)
    print("out", out.shape, out.dtype)


# revision 85
# speedup vs baseline: 1.0015x; 1.0015x over previous
"""Fused pre-LN multi-head attention block for Trainium2, 8-core data parallel.

Problem (per batch element, B=8, one batch element per core):
  x: [2048, 128] f32
  pre-LN -> QKV -> heads via a RAW reshape (B,S,D)->(B,H,S',dh): head h is a
  256-token slab; its 2048 "subtokens" are (token, feature-group) pairs:
  subtoken 8*t+g of head h = features [16g:16g+16) of token 256h+t.
  energy = Qv @ Kv^T per head; softmax; /sqrt(128) AFTER softmax; O = A @ Vv;
  raw reshape back; out = O @ Wo^T + x.

v4 design (pipeline-overlap driven; sim 290us -> 190us vs the v2 baseline):
  - Subtokens processed in block-major kappa order (kappa = 256*g + t), legal
    because softmax+AV are k-permutation invariant and the q-permutation is
    undone at output assembly.
  - Q/K stacks via natural-layout Q^T/K^T half-tensors: matmuls produce
    QT_half [4 g-blocks x (16 data + 16 zero) rows, tokens] (full-width
    PSUM->SBUF copies, bf16), then the (g: partition -> free) band shuffle
    runs as 32-aligned [32, 256] SBUF->SBUF copies on the otherwise-idle
    Pool engine (head 0's copies on Act/DVE for early start). This removes
    the 32-partition-wide stack copies (~30us of Act/DVE) of the v2 design;
    the zero pad rows ride along in the 32-row copies for free.
  - Energy: bf16 matmuls 512 wide (1 cycle/row), lhsT = KST 32-row
    zero-padded blocks via tile_position row banding.
  - exp is the wall (8*2048^2 = 33.5M elements/core on two engines): split
    between Act (true Exp) and DVE (Schraudolph exp: bf16 bits =
    int16(x*128/ln2 + (127-c)*128), ~3% rel err, irrelevant after softmax
    normalization) in 1024-col chunks with a 3-deep PSUM ping-pong (6 banks)
    so both engines run concurrently and the PE never waits for a free bank.
    A greedy balancer with tuned per-instruction fixed costs (Act ~365ns:
    222-cycle SBUF access + seq dispatch with exec-queue depth 0; DVE
    ~200ns with its depth-8 exec queue) assigns every flexible op; the
    equilibrium runs DVE ~88% busy and Act ~81%.
  - AV flipped: A (SBUF, bf16) is the *stationary* operand, the 17-wide
    [V|1] block is moving (17 rows/matmul, output in natural q orientation).
  - Ones column rides the AV matmul for deferred softmax normalization;
    1/sqrt(128) is folded into Wo host-side. Finalize normalizes both
    par-halves with a single reciprocal + tensor_tensor per head.
  - LayerNorm stats via bn_stats/bn_aggr (DVE), xn application on the idle
    Pool engine; x DMA'd in 4 per-LN-group tiles issued ahead of the weight
    DMAs so the first energy matmul lands ~6.5us in. Second-half LN, Q^T/K^T,
    X=1 shuffles and V projections drip into the first main-loop units.
  - AV lags exp by 3 units and is emitted before each unit's exp chunks, so
    the drain after the last exp is one AV burst + one finalize + DMA out.
"""

import sys

if "/opt/trn_rl_repo" not in sys.path:
    sys.path.insert(0, "/opt/trn_rl_repo")

from contextlib import ExitStack

import numpy as np

import concourse.bacc as bacc
import concourse.mybir as mybir
from concourse import masks
from concourse.tile import TileContext
from concourse.bass_utils import run_bass_kernel_spmd

S = 2048
D = 128
H = 8
DH = 16
NT = S // 128           # 16 s-tiles / k-tiles per batch element
VBLK = 17 * H           # 136 cols per s-tile block in v_sb
NQC = 4                 # 512-wide q chunks per head
NCH = 8                 # 1024-col exp chunks per unit (2 k-tiles each)
EPS = 1e-5
F32 = mybir.dt.float32
F32R = mybir.dt.float32r
BF16 = mybir.dt.bfloat16
I16 = mybir.dt.int16
AX = mybir.AluOpType
AF = mybir.ActivationFunctionType

LN2 = float(np.log(2.0))
SCH_A = 128.0 / LN2
SCH_B = (127.0 - 0.0579) * 128.0


class Balancer:
    """Greedy Act/DVE assignment for flexible ops (exp chunks, psum copies)."""

    ACT_NS = 0.8333
    DVE_NS = 1.0417

    def __init__(self):
        self.t = {"act": 0.0, "dve": 0.0}

    def charge(self, eng, cols, fixed=150.0):
        rate = self.ACT_NS if eng == "act" else self.DVE_NS
        self.t[eng] += cols * rate + fixed

    def pick(self, cols, dve_cols=None):
        fin_act = self.t["act"] + cols * self.ACT_NS + 365.0
        fin_dve = (
            self.t["dve"]
            + (cols if dve_cols is None else dve_cols) * self.DVE_NS
            + 200.0
        )
        if fin_act <= fin_dve:
            self.t["act"] = fin_act
            return "act"
        self.t["dve"] = fin_dve
        return "dve"


def build_mha_kernel(tc, outs, ins):
    nc = tc.nc
    x_d = ins["x"]          # [2048, 128] natural
    wv2_d = ins["wv2"]      # [128, 256] = [(Wv*ln_w).T | dup]
    wq2_d = ins["wq2"]      # [128, 256] 2 halves x 4 zero-padded g-blocks
    wk2_d = ins["wk2"]      # [128, 256] same for Wk
    wo_d = ins["wo"]        # [128, 128] = Wo.T / sqrt(D)
    out_d = outs["out_t"]   # [128, 2048] out^T (residual added host-side)

    bal = Balancer()

    ctx = ExitStack()
    const = ctx.enter_context(tc.tile_pool(name="const", bufs=1))
    iop = ctx.enter_context(tc.tile_pool(name="io", bufs=1))
    workp = ctx.enter_context(tc.tile_pool(name="work", bufs=2))
    psum = ctx.enter_context(tc.tile_pool(name="psum", bufs=1, space="PSUM"))

    def flex_copy(out_ap, in_ap, cols, dve_cols=None):
        # dve_cols: effective DVE cost override (bf16 SBUF->SBUF copies run
        # in the 4x_2p DVE mode, so their DVE cost is cols/4)
        if bal.pick(cols, dve_cols) == "act":
            nc.scalar.activation(out_ap, in_ap, AF.Copy)
        else:
            nc.vector.tensor_copy(out_ap, in_ap)

    def flex_exp(out_ap, in_ap, cols, force=None):
        eng = force if force is not None else bal.pick(cols)
        if force is not None:
            bal.charge(force, cols)
        if eng == "act":
            nc.scalar.activation(out_ap, in_ap, AF.Exp)
        else:
            nc.vector.tensor_scalar(
                out=out_ap.bitcast(I16), in0=in_ap,
                scalar1=SCH_A, scalar2=SCH_B, op0=AX.mult, op1=AX.add,
            )

    # ---- big persistent SBUF tensors ---------------------------------------
    # x natural [p, (t d)], split into 4 tiles so LayerNorm group q depends
    # only on its own DMA chunk; x DMAs are issued FIRST so the weight DMAs
    # don't serialize ahead of them on the HWDGE queue.
    x_sb4 = [
        iop.tile([128, 512], F32, tag=f"x{q}", name=f"x{q}") for q in range(4)
    ]
    for q in range(4):
        nc.sync.dma_start(
            out=x_sb4[q][:].rearrange("p (t d) -> p t d", d=D),
            in_=x_d.rearrange("(t p) d -> p t d", p=128)[:, 4 * q:4 * q + 4, :],
        )

    def x_tile(st):
        return x_sb4[st // 4][:, (st % 4) * D:(st % 4 + 1) * D]

    # ---- constants / weights (DMA'd after x; first needed ~7us in) --------
    ident = const.tile([128, 128], F32, tag="ident")
    masks.make_identity(nc, ident[:])
    ident_r = const.tile([128, 128], F32R, tag="identr")
    nc.vector.tensor_copy(ident_r[:], ident[:])
    wq2_sb = const.tile([128, 256], F32R, tag="wq2")
    nc.sync.dma_start(out=wq2_sb[:], in_=wq2_d[:])
    wk2_sb = const.tile([128, 256], F32R, tag="wk2")
    nc.sync.dma_start(out=wk2_sb[:], in_=wk2_d[:])
    wv2_sb = const.tile([128, 256], F32R, tag="wv2")
    nc.sync.dma_start(out=wv2_sb[:], in_=wv2_d[:])
    wo_sb = const.tile([128, 128], F32R, tag="wo")
    nc.sync.dma_start(out=wo_sb[:], in_=wo_d[:])
    wob_sb = const.tile([128, 128], BF16, tag="wob")
    nc.gpsimd.tensor_copy(wob_sb[:], wo_sb[:])
    xnt_sb = iop.tile([128, S], F32R, tag="xnt")      # xn^T   [d, s]
    v_sb = iop.tile([128, NT * VBLK], BF16, tag="v")  # [V|1] blocks per s-tile
    qst = [iop.tile([128, S], BF16, tag=f"qst{i}", name=f"qst{i}") for i in range(2)]
    kst = [iop.tile([128, S], BF16, tag=f"kst{i}", name=f"kst{i}") for i in range(2)]
    # Q^T/K^T halves: [4 g-blocks x (16 ch + 16 zero) rows, 2048 tokens]
    qth = [iop.tile([128, S], BF16, tag=f"qth{i}", name=f"qth{i}") for i in range(2)]
    kth = [iop.tile([128, S], BF16, tag=f"kth{i}", name=f"kth{i}") for i in range(2)]
    otT_sb = iop.tile([128, S], BF16, tag="otT")      # O^T (normalized)
    outt_sb = iop.tile([128, S], F32, tag="outt")     # final out^T

    ones_sb = const.tile([128, 8], F32, tag="ones")
    nc.gpsimd.memset(ones_sb[:], 1.0)

    def emit_ones():
        # v_sb ones columns; deferred so the Pool stream reaches the first
        # xn applications before this (only needed by the first AV, iter 2)
        for st in range(NT):
            nc.gpsimd.tensor_copy(
                v_sb[:, st * VBLK:(st + 1) * VBLK]
                .rearrange("p (g c) -> p g c", c=17)[:, :, 16:17],
                ones_sb[:].rearrange("p (g c) -> p g c", c=1),
            )

    # ---- LayerNorm (stats on DVE via bn_stats, apply on Pool) + xn^T -------
    stats = const.tile([128, 64], F32, tag="stats")
    mv = stats[:, 0:32].rearrange("p (t c) -> p t c", c=2)   # (mean, var)
    rstd = stats[:, 32:48]
    mrs = stats[:, 48:64]

    def emit_ln_stats(g4):
        s4 = slice(4 * g4, 4 * g4 + 4)
        bn6 = workp.tile([128, 24], F32, tag="bn6", bufs=2)
        for j in range(4):
            st = 4 * g4 + j
            nc.vector.bn_stats(
                out=bn6[:, 6 * j:6 * (j + 1)],
                in_=x_tile(st),
            )
            bal.charge("dve", 128)
            nc.vector.bn_aggr(out=mv[:, st, :], in_=bn6[:, 6 * j:6 * (j + 1)])
            bal.charge("dve", 6)
        # rstd = sqrt(1/(var+eps)); mrs = mean*rstd
        var_ap = mv[:, s4, 1:2]
        nc.vector.tensor_scalar_add(rstd[:, s4], var_ap.opt(), EPS)
        nc.vector.reciprocal(rstd[:, s4], rstd[:, s4])
        bal.charge("dve", 8)
        nc.scalar.activation(rstd[:, s4], rstd[:, s4], AF.Sqrt)
        bal.charge("act", 4)
        nc.gpsimd.tensor_mul(mrs[:, s4], mv[:, s4, 0:1].opt(), rstd[:, s4])

    def emit_ln_apply(g4):
        pt = psum.tile([128, 512], F32R, tag="e", bufs=3)
        for j in range(4):
            st = 4 * g4 + j
            xn = workp.tile([128, 128], F32R, tag="xn", bufs=8)
            nc.gpsimd.tensor_scalar(
                out=xn[:], in0=x_tile(st),
                scalar1=rstd[:, st:st + 1], scalar2=mrs[:, st:st + 1],
                op0=AX.mult, op1=AX.subtract,
            )
            nc.tensor.transpose(pt[:, j * 128:(j + 1) * 128], xn[:], ident_r[:])
        flex_copy(xnt_sb[:, g4 * 512:(g4 + 1) * 512], pt[:], 512)

    def emit_ln(g4):
        emit_ln_stats(g4)
        emit_ln_apply(g4)

    # ---- Q^T/K^T half-tensor production ------------------------------------
    # One matmul per (side, half, 512-token chunk): out rows 32a+c =
    # channel 16(4*half+a)+c for c<16, zero for c>=16.
    def emit_qt(side, half, tp):
        # tp: token pair index (0: tokens 0..1023, 1: 1024..2047)
        w_sb, dst = (wq2_sb, qth) if side == "q" else (wk2_sb, kth)
        qt = psum.tile([128, 1024], F32, tag="e", bufs=3, name="qt")
        for j in range(2):
            tok0 = tp * 1024 + j * 512
            nc.tensor.matmul(
                qt[:, j * 512:(j + 1) * 512],
                lhsT=w_sb[:, half * 128:(half + 1) * 128],
                rhs=xnt_sb[:, tok0:tok0 + 512],
                start=True, stop=True,
            )
        flex_copy(dst[half][:, tp * 1024:(tp + 1) * 1024], qt[:], 1024)

    # band shuffle: (qst|kst)[X][32m:32m+32, 256g:+256] =
    #               (qth|kth)[g//4][32(g%4):+32, 256(4X+m):+256]
    # (zero pad rows ride along). Pool for all but head 0 (early start).
    def emit_shuffle(X, m, side, g, on_pool):
        src = (qth if side == "q" else kth)[g // 4][
            32 * (g % 4):32 * (g % 4) + 32,
            256 * (4 * X + m):256 * (4 * X + m) + 256,
        ]
        dst = (qst if side == "q" else kst)[X][
            32 * m:32 * m + 32, 256 * g:256 * (g + 1)
        ]
        if on_pool:
            nc.gpsimd.tensor_copy(dst, src)
        else:
            flex_copy(dst, src, 256, dve_cols=64)

    def emit_v(sp):
        # two s-tiles' V projections in one PSUM tile, one strided copy
        vp = psum.tile([128, 512], F32, tag="e", bufs=3, name="vp")
        for j in range(2):
            st = 2 * sp + j
            nc.tensor.matmul(
                vp[:, j * 256:(j + 1) * 256],
                lhsT=xnt_sb[:, st * 128:(st + 1) * 128],
                rhs=wv2_sb[:],
                start=True, stop=True,
            )
        flex_copy(
            v_sb[:, 2 * sp * VBLK:(2 * sp + 2) * VBLK]
            .rearrange("p (s g c) -> p s g c", s=2, c=17)[:, :, :, 0:16],
            vp[:].rearrange("p (s d g c) -> p s d g c", s=2, d=2, c=16)
            [:, :, 0, :, :],
            256,
        )

    # ---- preamble orchestration --------------------------------------------
    # Upfront: only what unit 0 (head 0) needs — LN of tokens 0..1023,
    # Q^T/K^T for those tokens, head 0's shuffle (flex engines), V0, and the
    # X0 m1-3 Pool shuffles (Pool doesn't block the exp engines). LN of the
    # second half, Q^T/K^T-B, X1 Pool shuffles, and V1..7 drip into the
    # first main-loop units (deadline: head m of X at unit 4*(4X+m)).
    emit_ln(0)
    emit_ln(1)
    emit_ones()
    for half in range(2):
        emit_qt("k", half, 0)
    for half in range(2):
        emit_qt("q", half, 0)
    for g in range(H):
        emit_shuffle(0, 0, "k", g, on_pool=False)
    for g in range(H):
        emit_shuffle(0, 0, "q", g, on_pool=False)
    emit_v(0)
    for m in range(1, 4):
        for g in range(H):
            emit_shuffle(0, m, "k", g, on_pool=True)
        for g in range(H):
            emit_shuffle(0, m, "q", g, on_pool=True)

    def emit_qtb_shuffles():
        for m in range(4):
            for g in range(H):
                emit_shuffle(1, m, "k", g, on_pool=True)
            for g in range(H):
                emit_shuffle(1, m, "q", g, on_pool=True)

    late_work = (
        [("ln", 2), ("ln", 3),
         ("qt", "k", 0, 1), ("qt", "k", 1, 1),
         ("qt", "q", 0, 1), ("qt", "q", 1, 1),
         ("shufB",), ("v", 1)]
        + [("v", sp) for sp in range(2, 8)]
    )

    def emit_late(w):
        if w[0] == "ln":
            emit_ln(w[1])
        elif w[0] == "qt":
            emit_qt(w[1], w[2], w[3])
        elif w[0] == "shufB":
            emit_qtb_shuffles()
        else:
            emit_v(w[1])

    # ---- attention main loop (software-pipelined: AV lags E/exp by 3) ------
    av_tile = {}
    pend = []

    def emit_e_exp(h, qc, last=False):
        X, m = h // 4, h % 4
        a_sb = workp.tile([128, NT * 512], BF16, tag="a", bufs=8)
        for c in range(NCH):
            et = psum.tile([128, 1024], F32, tag="e", bufs=3)
            for j in range(2):
                kt = 2 * c + j
                nc.tensor.matmul(
                    et[:, j * 512:(j + 1) * 512],
                    lhsT=kst[X][32 * m:32 * m + 32, kt * 128:(kt + 1) * 128],
                    rhs=qst[X][32 * m:32 * m + 32,
                               qc * 512:(qc + 1) * 512],
                    start=True, stop=True,
                    tile_position=(32 * m, 0),
                )
            # on the very last unit, pin the final two chunks to opposite
            # engines so the drain (AV -> finalize -> DMA) starts earliest
            force = None
            if last and c >= NCH - 2:
                force = "dve" if c == NCH - 2 else "act"
            flex_exp(a_sb[:, c * 1024:(c + 1) * 1024], et[:], 1024, force)
        return a_sb

    def emit_av(h, qc, a_sb):
        if qc == 0:
            av_tile[h] = psum.tile([128, 512], F32, tag="av", bufs=2, name="avbank")
        bank = av_tile[h]
        for qb in range(4):
            slot = 4 * qc + qb
            for kt in range(NT):
                st = 2 * h + (kt % 2)
                gk = kt // 2
                nc.tensor.matmul(
                    bank[:, slot * 17:slot * 17 + 17],
                    lhsT=a_sb[:, kt * 512 + qb * 128:kt * 512 + (qb + 1) * 128],
                    rhs=v_sb[:, st * VBLK + 17 * gk:st * VBLK + 17 * gk + 17],
                    start=(kt == 0), stop=(kt == NT - 1),
                )

    def emit_finalize(h):
        # split=True (last head): two independent per-par chains so the
        # copy -> Wo -> copy -> DMA latency pipelines at the kernel tail.
        split = False
        bank = av_tile.pop(h)
        # one reciprocal + one normalize covering BOTH par-halves; the output
        # AP de-interleaves slots so each par's 128 cols land contiguous
        slots = bank[:, 0:272].rearrange("p (q par c) -> p q par c", par=2, c=17)
        rec = workp.tile([128, 16], F32, tag="rec", bufs=2)
        nc.vector.reciprocal(rec[:], slots[:, :, :, 16:17].opt())
        bal.charge("dve", 16)
        o_nat = workp.tile([128, 256], F32R, tag="on", bufs=2)
        nc.vector.tensor_tensor(
            out=o_nat[:].rearrange("p (par q c) -> p q par c", par=2, c=DH),
            in0=slots[:, :, :, 0:16],
            in1=rec[:].rearrange("p (q par) -> p q par", par=2)
            .broadcast_to((128, 8, 2, DH)),
            op=AX.mult,
        )
        bal.charge("dve", 256)
        onats = [o_nat[:, 0:128], o_nat[:, 128:256]]
        fin = psum.tile([128, 512], F32R, tag="av", bufs=2, name="fin")

        def tail(lo, hi):
            cols = hi - lo
            o0 = 256 * h + lo
            flex_copy(otT_sb[:, o0:o0 + cols], fin[:, lo:lo + cols], cols)
            nc.tensor.matmul(
                fin[:, 256 + lo:256 + hi].bitcast(F32),
                lhsT=wob_sb[:],
                rhs=otT_sb[:, o0:o0 + cols],
                start=True, stop=True,
            )
            flex_copy(outt_sb[:, o0:o0 + cols],
                      fin[:, 256 + lo:256 + hi].bitcast(F32), cols)
            nc.sync.dma_start(
                out=out_d[:, o0:o0 + cols],
                in_=outt_sb[:, o0:o0 + cols],
            )

        if split:
            for par in range(2):
                nc.tensor.transpose(
                    fin[:, par * 128:(par + 1) * 128], onats[par], ident_r[:]
                )
                tail(par * 128, (par + 1) * 128)
        else:
            for par in range(2):
                nc.tensor.transpose(
                    fin[:, par * 128:(par + 1) * 128], onats[par], ident_r[:]
                )
            tail(0, 256)

    for u in range(H * NQC):
        h, qc = divmod(u, NQC)
        # AV lags exp by 2; emitted BEFORE the unit's exp so finalize's
        # DVE work queues ahead of the next exp chunks (frees the av bank
        # sooner for the next head's accumulator).
        if len(pend) >= 3:
            u2 = pend.pop(0)
            emit_av(*u2)
            if u2[1] == NQC - 1:
                emit_finalize(u2[0])
        a_sb = emit_e_exp(h, qc, last=(u == H * NQC - 1))
        # drip-feed two late-preamble pieces per early unit
        for _ in range(2):
            if late_work:
                emit_late(late_work.pop(0))
        pend.append((h, qc, a_sb))
    while pend:
        u2 = pend.pop(0)
        emit_av(*u2)
        if u2[1] == NQC - 1:
            emit_finalize(u2[0])

    ctx.close()


def host_prep(x, ln_w, ln_b, Wq, bq, Wk, bk, Wv, bv, Wo, bo):
    """Fold LN affine into weights; build combined zero-padded Q/K g-blocks."""
    for name, b in (("ln_b", ln_b), ("bq", bq), ("bk", bk), ("bv", bv), ("bo", bo)):
        assert np.abs(np.asarray(b)).max() == 0.0, f"{name} must be zero"
    ln_w = np.asarray(ln_w, np.float32)
    Wq_e = np.asarray(Wq, np.float32) * ln_w[None, :]
    Wk_e = np.asarray(Wk, np.float32) * ln_w[None, :]
    Wv_e = np.asarray(Wv, np.float32) * ln_w[None, :]

    # [128, 2*128]: half h, block a = [W.T cols 16(4h+a):+16 | 16 zeros]
    def halves(W):
        out = np.zeros((D, 256), np.float32)
        WT = W.T
        for h in range(2):
            for a in range(4):
                g = 4 * h + a
                out[:, h * 128 + a * 32:h * 128 + a * 32 + 16] = WT[
                    :, 16 * g:16 * (g + 1)
                ]
        return out

    wq2 = halves(Wq_e)
    wk2 = halves(Wk_e)
    wv2 = np.concatenate([Wv_e.T, Wv_e.T], axis=1).astype(np.float32)
    wv2 = np.ascontiguousarray(wv2)
    wo = np.ascontiguousarray(np.asarray(Wo, np.float32).T / np.sqrt(np.float32(D)))
    return wv2, wq2, wk2, wo


_CACHED = {}


def _build_nc():
    nc = bacc.Bacc("TRN2", target_bir_lowering=False, debug=False, num_devices=8)
    x_in = nc.dram_tensor("x", [S, D], F32, kind="ExternalInput").ap()
    wv2_in = nc.dram_tensor("wv2", [D, 256], F32R, kind="ExternalInput").ap()
    wq2_in = nc.dram_tensor("wq2", [D, 256], F32R, kind="ExternalInput").ap()
    wk2_in = nc.dram_tensor("wk2", [D, 256], F32R, kind="ExternalInput").ap()
    wo_in = nc.dram_tensor("wo", [D, D], F32R, kind="ExternalInput").ap()
    out_t = nc.dram_tensor("out_t", [D, S], F32, kind="ExternalOutput").ap()
    with TileContext(nc) as tc:
        build_mha_kernel(
            tc,
            {"out_t": out_t},
            {"x": x_in, "wv2": wv2_in, "wq2": wq2_in, "wk2": wk2_in,
             "wo": wo_in},
        )
    nc.compile()
    return nc


def kernel(x, ln_w, ln_b, Wq, bq, Wk, bk, Wv, bv, Wo, bo):
    x = np.asarray(x, np.float32)
    B = x.shape[0]
    assert x.shape == (B, S, D) and B == 8
    wv2, wq2, wk2, wo = host_prep(x, ln_w, ln_b, Wq, bq, Wk, bk, Wv, bv, Wo, bo)

    if "nc" not in _CACHED:
        _CACHED["nc"] = _build_nc()
    nc = _CACHED["nc"]

    in_maps = [
        {
            "x": np.ascontiguousarray(x[c]),
            "wv2": wv2,
            "wq2": wq2,
            "wk2": wk2,
            "wo": wo,
        }
        for c in range(B)
    ]
    res = run_bass_kernel_spmd(nc, in_maps, core_ids=list(range(B)))
    out = np.stack([res.results[c]["out_t"].T for c in range(B)]) + x
    return out.astype(np.float32)


if __name__ == "__main__":
    rng = np.random.default_rng(0)
    x = rng.standard_normal((8, S, D), dtype=np.float32)
    ln_w = np.ones(D, np.float32)
    z = np.zeros(D, np.float32)
    s = 1.0 / np.sqrt(D)
    Wq = rng.standard_normal((D, D), dtype=np.float32) * s
    Wk = rng.standard_normal((D, D), dtype=np.float32) * s
    Wv = rng.standard_normal((D, D), dtype=np.float32) * s
    Wo = rng.standard_normal((D, D), dtype=np.float32) * s
    out = kernel(x, ln_w, z, Wq, z, Wk, z, Wv, z, Wo, z)
    print("out", out.shape, out.dtype)
